# revision 12
# baseline (speedup 1.0000x reference)
"""GCNConv (N=100000 nodes, d=64, E=1.6M edges) on 8 Trainium2 NeuronCores.

Formula (DGL GraphConv, in==out feats):
    out_deg = bincount(src); in_deg = bincount(dst)
    norm_src = clip(out_deg,1)^-0.5 ; norm_dst = clip(in_deg,1)^-0.5
    feat = x * norm_src[:,None]
    agg[d] = sum_{e: dst[e]=d} feat[src[e]]
    out = (agg * norm_dst[:,None]) @ W

Distribution: nodes sharded 8 ways (12500/core).
  Phase 1 (core k, edges with src in shard k): out-degree histogram over
    32-node windows: one-hot (DVE is_equal), per-window free-axis reduce,
    one tiny [32,1] matmul per window writing a COLUMN of a packed
    [128, 98] PSUM degree map (partition offset 32*(w%4), free offset
    w//4); one batched sqrt(+eps)/reciprocal pass gives norm_src for the
    whole shard at once; x loaded and feat stored 4 blocks per DMA.
    feat shard is [12500, 65] bf16 (col 64 = 1.0, gives in-degree for
    free in phase 2).
  AllGather feat shards -> full gather table [100000, 65] per core.
  Phase 2 (core k, edges with dst in shard k, bucketed per 128-node
    block): ONE batched indirect-DMA per 32-tile chunk gathers 32*128
    feat rows (offset AP [128, 32], dest [128, 32*65]) -- the SWDGE
    fixed cost (~1us) amortizes over 32 tiles instead of being paid per
    tile; per 128-edge tile one one-hot scatter matmul into PSUM
    aggT [65, 128] (row 64 = in_deg); per block: transpose the raw
    degree row via PE, columnar sqrt(+eps)/reciprocal (parallel across
    partitions), out_blk = aggT[:64].T @ W, row-scale.

deg==0 handling: sqrt bias eps keeps norm finite (1000); such rows are
never gathered (out-deg) / are exactly zero (in-deg), so the value never
matters and no clip is needed.

Host side only shards/buckets edges and builds index/window inputs; all
arithmetic of the formula (degrees, norms, scaling, aggregation, matmul)
runs on device.
"""

import sys

if "/opt/trn_rl_repo" not in sys.path:
    sys.path.insert(0, "/opt/trn_rl_repo")

import numpy as np

import concourse.bass as bass
import concourse.mybir as mybir
import concourse.tile as tile

N_NODES = 100000
D = 64
N_CORES = 8
SHARD = N_NODES // N_CORES  # 12500
W1 = 32  # phase-1 (degree-count) window width
W2 = 128  # phase-2 window width == node block
P = 128  # edges per tile (matmul contraction dim)
CHUNK1 = 64  # phase-1 max tiles per chunk (window-aligned packing)
CHUNK2 = 1  # phase-2 tiles per chunk == rows per batched indirect gather
NBLK = (SHARD + P - 1) // P  # 98 node blocks per shard
EPS = 1e-6

F32 = mybir.dt.float32
BF16 = mybir.dt.bfloat16
I32 = mybir.dt.int32

PRECISION = "bf16"  # "bf16" | "fp32" message/table dtype


def split_waits(nc, maxw=1):
    """This walrus build allows at most `maxw` sem-waits per instruction;
    move extras onto preceding InstEventSemaphore carriers (same engine)."""
    for f in nc.m.functions:
        for blk in f.blocks:
            newl = []
            for ins in blk.instructions:
                si = ins.sync_info
                if si is not None and si.on_wait and len(si.on_wait) > maxw:
                    waits = list(si.on_wait)
                    carry, keep = waits[:-maxw], waits[-maxw:]
                    for i in range(0, len(carry), maxw):
                        w = mybir.InstEventSemaphore(
                            name=nc.get_next_instruction_name(), ins=[], outs=[]
                        )
                        w.engine = ins.engine
                        w.sync_info = mybir.SyncInfo(
                            on_wait=carry[i : i + maxw], on_update=[]
                        )
                        newl.append(w)
                    ins.sync_info = mybir.SyncInfo(
                        on_wait=keep, on_update=list(si.on_update)
                    )
                newl.append(ins)
            blk.instructions[:] = newl


def _layout(cnts_per_core):
    """Uniform (max-over-cores) tiles per window."""
    tiles_w = (cnts_per_core.max(axis=0) + P - 1) // P
    tbase = np.concatenate([[0], np.cumsum(tiles_w)[:-1]])
    return tiles_w.astype(np.int64), tbase.astype(np.int64), int(tiles_w.sum())


def _prep(x, W, src, dst):
    """Host-side sharding: bucket edges by shard and window, build per-core
    device inputs and the shared (uniform across cores) tile metadata."""
    src = np.asarray(src)
    dst = np.asarray(dst)
    x = np.asarray(x, dtype=np.float32)
    W = np.asarray(W, dtype=np.float32)

    nwin1 = (SHARD + W1 - 1) // W1
    nwin2 = (SHARD + W2 - 1) // W2

    per_core = []
    c1 = np.zeros((N_CORES, nwin1), dtype=np.int64)
    c2 = np.zeros((N_CORES, nwin2), dtype=np.int64)
    for k in range(N_CORES):
        sel1 = (src // SHARD) == k
        loc1 = src[sel1] - SHARD * k
        w1 = loc1 // W1
        c1[k] = np.bincount(w1, minlength=nwin1)

        sel2 = (dst // SHARD) == k
        loc2 = dst[sel2] - SHARD * k
        gidx = src[sel2]
        w2 = loc2 // W2
        c2[k] = np.bincount(w2, minlength=nwin2)
        per_core.append((loc1, w1, loc2, w2, gidx))

    t1_w, t1_base, T1 = _layout(c1)
    t2_w, t2_base, T2 = _layout(c2)

    import ml_dtypes

    mnp = np.float32 if PRECISION == "fp32" else ml_dtypes.bfloat16

    w64 = W.astype(mnp)
    iota1 = np.broadcast_to(np.arange(W1, dtype=np.float32), (P, W1)).copy()
    iota2 = np.broadcast_to(np.arange(W2, dtype=np.float32), (P, W2)).copy()
    ones = np.ones((P, 1), dtype=np.float32)
    ones_m = np.ones((P, 1), dtype=mnp)

    ins_maps = []
    for k in range(N_CORES):
        loc1, w1, loc2, w2, gidx = per_core[k]

        def fill(loc, wv, base, T, win, payload=None):
            order = np.argsort(wv, kind="stable")
            ws = wv[order]
            cnt = np.bincount(wv, minlength=len(base))
            starts = np.concatenate([[0], np.cumsum(cnt)[:-1]])
            rank = np.arange(len(order)) - starts[ws]
            col = base[ws] + rank // P
            lane = rank % P
            arr = np.full((P, T), float(win), dtype=np.float32)
            arr[lane, col] = (loc[order] - win * ws).astype(np.float32)
            parr = None
            if payload is not None:
                parr = np.zeros((P, T), dtype=np.int32)
                parr[lane, col] = payload[order].astype(np.int32)
            return arr, parr

        p1win, _ = fill(loc1, w1, t1_base, T1, W1)
        p2win, p2idx = fill(loc2, w2, t2_base, T2, W2, payload=gidx)

        ins_maps.append(
            {
                "xs": np.ascontiguousarray(x[SHARD * k : SHARD * (k + 1)]),
                "p1win": p1win,
                "p2idx": p2idx,
                "p2win": p2win,
                "w64": w64,
                "iota1": iota1,
                "iota2": iota2,
                "ones": ones,
                "ones_m": ones_m,
            }
        )

    meta = {
        "T1": T1,
        "T2": T2,
        "t1_w": t1_w,
        "t2_w": t2_w,
        "nwin1": nwin1,
        "nwin2": nwin2,
    }
    return ins_maps, meta


def _tile_maps(meta):
    # phase-1: pack whole windows into chunks of <= CHUNK1 tiles.
    # chunk entry: (t0, cw, [(w, a, b), ...]) with a/b tile offsets in chunk.
    chunks1 = []
    cur = []
    t0 = 0
    pos = 0
    for w, n in enumerate(meta["t1_w"]):
        n = int(n)
        if n == 0:
            continue
        if pos + n > CHUNK1 and cur:
            chunks1.append((t0, pos, cur))
            t0 += pos
            pos = 0
            cur = []
        cur.append((w, pos, pos + n))
        pos += n
    if cur:
        chunks1.append((t0, pos, cur))
    meta["p1_chunks"] = chunks1

    # phase-2 per-tile maps
    win_of_tile = []
    first_of_win = {}
    last_of_win = {}
    for w, n in enumerate(meta["t2_w"]):
        for i in range(int(n)):
            t = len(win_of_tile)
            if i == 0:
                first_of_win[w] = t
            last_of_win[w] = t
            win_of_tile.append(w)
    meta["p2_win_of_tile"] = win_of_tile
    meta["p2_first"] = first_of_win
    meta["p2_last"] = last_of_win
    return meta


def _build_nc(meta, do_split_waits=True):
    T1, T2 = meta["T1"], meta["T2"]
    MD = F32 if PRECISION == "fp32" else BF16

    nc = bass.Bass()
    xs = nc.declare_dram_parameter("xs", [SHARD, D], F32, isOutput=False)
    p1win_d = nc.declare_dram_parameter("p1win", [P, T1], F32, isOutput=False)
    p2idx_d = nc.declare_dram_parameter("p2idx", [P, T2], I32, isOutput=False)
    p2win_d = nc.declare_dram_parameter("p2win", [P, T2], F32, isOutput=False)
    w64_d = nc.declare_dram_parameter("w64", [D, D], MD, isOutput=False)
    iota1_d = nc.declare_dram_parameter("iota1", [P, W1], F32, isOutput=False)
    iota2_d = nc.declare_dram_parameter("iota2", [P, W2], F32, isOutput=False)
    ones_d = nc.declare_dram_parameter("ones", [P, 1], F32, isOutput=False)
    onesm_d = nc.declare_dram_parameter("ones_m", [P, 1], MD, isOutput=False)
    out_d = nc.declare_dram_parameter("out", [SHARD, D], F32, isOutput=True)

    feat_s = nc.dram_tensor("feat_s", [SHARD, D + 1], MD)
    feat_f = nc.dram_tensor("feat_f", [N_NODES, D + 1], MD)

    with tile.TileContext(nc) as tc:
        with tc.tile_pool(name="consts", bufs=1) as consts:
            w64_sb = consts.tile([D, D], MD, tag="w64")
            iota1_sb = consts.tile([P, W1], F32, tag="iota1")
            iota2_sb = consts.tile([P, W2], F32, tag="iota2")
            ones_sb = consts.tile([P, 1], F32, tag="ones")
            onesm_sb = consts.tile([P, 1], MD, tag="onesm")
            nc.sync.dma_start(out=w64_sb[:], in_=w64_d[:])
            nc.sync.dma_start(out=iota1_sb[:], in_=iota1_d[:])
            nc.sync.dma_start(out=iota2_sb[:], in_=iota2_d[:])
            nc.sync.dma_start(out=ones_sb[:], in_=ones_d[:])
            nc.sync.dma_start(out=onesm_sb[:], in_=onesm_d[:])

            # ---------------- phase 1: out-degree -> feat shard -------------
            # x block loads: 4 blocks per DMA, all prefetched up front.
            n_xg = (NBLK - 2 + 3) // 4 + 2  # 24 groups of 4 + blocks 96, 97
            with (
                tc.tile_pool(name="p1x", bufs=1) as p_x,
                tc.tile_pool(name="p1win", bufs=2) as p_win,
                tc.tile_pool(name="p1oh", bufs=2) as p_oh,
                tc.tile_pool(name="p1s", bufs=4) as p_s,
                tc.tile_pool(name="p1deg", bufs=1, space="PSUM") as p_deg,
                tc.tile_pool(name="p1n", bufs=1) as p_n,
                tc.tile_pool(name="p1feat", bufs=4) as p_feat,
            ):
                xq_tiles = []
                for g in range(24):  # blocks 4g..4g+3, all full
                    xq = p_x.tile([P, 4, D], F32, tag=f"xq{g}")
                    nc.sync.dma_start(
                        out=xq[:],
                        in_=xs[P * 4 * g : P * 4 * (g + 1), :].rearrange(
                            "(a p) d -> p a d", p=P
                        ),
                    )
                    xq_tiles.append(xq)
                xb96 = p_x.tile([P, D], F32, tag="xb96")
                nc.sync.dma_start(out=xb96[:], in_=xs[P * 96 : P * 97, :])
                nb97 = SHARD - P * 97
                xb97 = p_x.tile([P, D], F32, tag="xb97")
                nc.sync.dma_start(out=xb97[:nb97], in_=xs[P * 97 : SHARD, :])

                ps_deg = p_deg.tile([P, NBLK], F32)
                nc.vector.memset(ps_deg[:], 0.0)

                # window pair (b, half) -> [64,1] matmul at base partition
                # 0 or 64 (PE disallows output base partition 96).
                t1_w = meta["t1_w"]
                nwin1 = meta["nwin1"]
                pair_present = {}
                for w in range(nwin1):
                    key = (w // 4, (w % 4) // 2)
                    pair_present.setdefault(key, set())
                    if int(t1_w[w]) > 0:
                        pair_present[key].add(w)
                pair_tiles = {}
                pair_done = {}

                for t0, cw, wins in meta["p1_chunks"]:
                    wt = p_win.tile([P, CHUNK1], F32, tag="wt")
                    nc.sync.dma_start(out=wt[:, :cw], in_=p1win_d[:, t0 : t0 + cw])
                    # transposed one-hot: [P, W1, cw]
                    oh = p_oh.tile([P, W1, CHUNK1], MD, tag="oh")
                    nc.vector.tensor_tensor(
                        out=oh[:, :, :cw],
                        in0=wt[:, None, :cw].to_broadcast([P, W1, cw]),
                        in1=iota1_sb[:, :, None].to_broadcast([P, W1, cw]),
                        op=mybir.AluOpType.is_equal,
                    )
                    for w, a, bnd in wins:
                        key = (w // 4, (w % 4) // 2)
                        if key not in pair_tiles:
                            pair_tiles[key] = p_s.tile(
                                [P, 2, W1], MD, name="S2", tag="S"
                            )
                            pair_done[key] = set()
                        S2 = pair_tiles[key]
                        with nc.allow_low_precision(
                            reason="window one-hot counts <= tiles/window, exact in bf16"
                        ):
                            nc.vector.tensor_reduce(
                                out=S2[:, w % 2, :, None],
                                in_=oh[:, :, a:bnd],
                                axis=mybir.AxisListType.X,
                                op=mybir.AluOpType.add,
                            )
                        pair_done[key].add(w)
                        if pair_done[key] == pair_present[key]:
                            b, half = key
                            for w2 in (4 * b + 2 * half, 4 * b + 2 * half + 1):
                                if w2 >= nwin1 or int(t1_w[w2]) == 0:
                                    nc.vector.memset(S2[:, w2 % 2, :], 0.0)
                            nc.tensor.matmul(
                                out=ps_deg[64 * half : 64 * half + 64, b : b + 1],
                                lhsT=S2.rearrange("p a b -> p (a b)"),
                                rhs=onesm_sb[:],
                                start=True,
                                stop=True,
                            )
                            del pair_tiles[key], pair_done[key]

                # batched norm_src: clip(deg,1) -> sqrt -> reciprocal, whole shard
                dcl1 = p_n.tile([P, NBLK], F32, tag="dcl1")
                nc.vector.tensor_scalar_max(dcl1[:], ps_deg[:], 1.0)
                sq1 = p_n.tile([P, NBLK], F32, tag="sq1")
                nc.scalar.sqrt(sq1[:], dcl1[:])
                norm1 = p_n.tile([P, NBLK], F32, tag="norm1")
                nc.vector.reciprocal(norm1[:], sq1[:])

                for g in range(24):
                    fbq = p_feat.tile([P, 4, D + 1], MD, tag="fbq")
                    nc.vector.tensor_mul(
                        fbq[:, :, 0:D],
                        xq_tiles[g][:],
                        norm1[:, 4 * g : 4 * g + 4][:, :, None].to_broadcast(
                            [P, 4, D]
                        ),
                    )
                    nc.vector.memset(fbq[:, :, D : D + 1], 1.0)
                    nc.sync.dma_start(
                        out=feat_s[P * 4 * g : P * 4 * (g + 1), :].rearrange(
                            "(a p) d -> p a d", p=P
                        ),
                        in_=fbq[:],
                    )
                for b, xb, nb in ((96, xb96, P), (97, xb97, nb97)):
                    fb = p_feat.tile([P, D + 1], MD, tag=f"fb{b}")
                    nc.vector.tensor_mul(
                        fb[:, 0:D], xb[:], norm1[:, b : b + 1].to_broadcast([P, D])
                    )
                    nc.vector.memset(fb[:, D : D + 1], 1.0)
                    nc.sync.dma_start(
                        out=feat_s[P * b : P * b + nb, :], in_=fb[:nb, :]
                    )

            # ---------------- allgather feat --------------------------------
            # Completion fence: Tile doesn't track the collective->gather RAW
            # dep through DRAM, so wait on an explicit semaphore inside a
            # critical section (Pool program order covers later gathers).
            ccsem = nc.alloc_semaphore("ccsem")
            with tc.tile_critical():
                nc.gpsimd.collective_compute(
                    "AllGather",
                    mybir.AluOpType.bypass,
                    replica_groups=[list(range(N_CORES))],
                    ins=[feat_s[:]],
                    outs=[feat_f[:]],
                ).then_inc(ccsem, 1)
                nc.gpsimd.wait_ge(ccsem, 1)

            # -------- phase 2: chunk gather + scatter matmul + W ------------
            with (
                tc.tile_pool(name="p2i", bufs=2) as p_idx,
                tc.tile_pool(name="p2w", bufs=2) as p_win2,
                tc.tile_pool(name="p2g", bufs=3) as p_g,
                tc.tile_pool(name="p2oh", bufs=2) as p_oh2,
                tc.tile_pool(name="p2ps", bufs=2, space="PSUM") as p_ps2,
                tc.tile_pool(name="p2tr", bufs=2, space="PSUM") as p_tr2,
                tc.tile_pool(name="p2ops", bufs=2, space="PSUM") as p_ops,
                tc.tile_pool(name="p2agg", bufs=2) as p_agg,
                tc.tile_pool(name="p2out", bufs=2) as p_out,
                tc.tile_pool(name="p2misc", bufs=4) as p_misc2,
            ):
                ps = None
                for c0 in range(0, T2, CHUNK2):
                    cw = min(CHUNK2, T2 - c0)
                    ix = p_idx.tile([P, CHUNK2], I32, tag="ix")
                    nc.sync.dma_start(out=ix[:, :cw], in_=p2idx_d[:, c0 : c0 + cw])
                    wt = p_win2.tile([P, CHUNK2], F32, tag="wt2")
                    nc.sync.dma_start(out=wt[:, :cw], in_=p2win_d[:, c0 : c0 + cw])
                    oh = p_oh2.tile([P, CHUNK2, W2], MD, tag="oh2")
                    nc.vector.tensor_tensor(
                        out=oh[:, :cw, :],
                        in0=wt[:, :cw, None].to_broadcast([P, cw, W2]),
                        in1=iota2_sb[:, None, :].to_broadcast([P, cw, W2]),
                        op=mybir.AluOpType.is_equal,
                    )
                    # one batched indirect gather for the whole chunk:
                    # row ix[p, j] of feat_f -> gb[p, j*65 : (j+1)*65]
                    gb = p_g.tile([P, CHUNK2 * (D + 1)], MD, tag="gb")
                    nc.gpsimd.indirect_dma_start(
                        out=gb[:, : cw * (D + 1)],
                        out_offset=None,
                        in_=feat_f[:],
                        in_offset=bass.IndirectOffsetOnAxis(
                            ap=ix[:, :cw], axis=0
                        ),
                    )
                    for j in range(cw):
                        t = c0 + j
                        b = meta["p2_win_of_tile"][t]  # window == block
                        if ps is None:
                            ps = p_ps2.tile([D + 1, P], F32)
                        nc.tensor.matmul(
                            out=ps[:],
                            lhsT=gb[:, (D + 1) * j : (D + 1) * (j + 1)],
                            rhs=oh[:, j, :],
                            start=(t == meta["p2_first"][b]),
                            stop=(t == meta["p2_last"][b]),
                        )
                        if t == meta["p2_last"][b]:
                            # raw in-degree row -> SBUF (ACT), transpose via
                            # PE, then columnar sqrt(+eps)/reciprocal
                            dc = p_misc2.tile([1, P], F32, tag="dc")
                            nc.scalar.copy(dc[:], ps[D : D + 1, :])
                            tp2 = p_tr2.tile([P, 1], F32)
                            nc.tensor.matmul(
                                out=tp2[:],
                                lhsT=dc[:],
                                rhs=ones_sb[0:1, 0:1],
                                start=True,
                                stop=True,
                            )
                            dmx = p_misc2.tile([P, 1], F32, tag="dmx")
                            nc.vector.tensor_scalar_max(dmx[:], tp2[:], 1.0)
                            sq2 = p_misc2.tile([P, 1], F32, tag="sq2")
                            nc.scalar.sqrt(sq2[:], dmx[:])
                            ncol2 = p_misc2.tile([P, 1], F32, tag="ncol2")
                            nc.vector.reciprocal(ncol2[:], sq2[:])
                            ag = p_agg.tile([D, P], MD, tag="ag")
                            nc.vector.tensor_copy(ag[:], ps[0:D, :])
                            op = p_ops.tile([P, D], F32)
                            nc.tensor.matmul(
                                out=op[:],
                                lhsT=ag[:],
                                rhs=w64_sb[:],
                                start=True,
                                stop=True,
                            )
                            ob = p_out.tile([P, D], F32, tag="ob")
                            nc.vector.tensor_mul(
                                ob[:], op[:], ncol2[:].to_broadcast([P, D])
                            )
                            nb = min(P, SHARD - P * b)
                            nc.sync.dma_start(
                                out=out_d[P * b : P * b + nb, :], in_=ob[:nb, :]
                            )
                            ps = None

    if do_split_waits:
        split_waits(nc)
    return nc


def kernel(x, W, src, dst):
    from concourse.bass_utils import run_bass_kernel_spmd

    ins_maps, meta = _prep(x, W, src, dst)
    meta = _tile_maps(meta)
    nc = _build_nc(meta)
    res = run_bass_kernel_spmd(nc, ins_maps, list(range(N_CORES)))
    out = np.concatenate([res.results[k]["out"] for k in range(N_CORES)], axis=0)
    return out.astype(np.float32)
